# revision 16
# baseline (speedup 1.0000x reference)
"""Trainium2 Bass kernel for the embedding_lookup Classifier problem.

Computation (per token t):
    out[t] = relu(W1[:VOCAB][tk[t]] + hs0[t] @ W1[VOCAB:] + b1) @ W2 + b2

Sharding: data-parallel over the batch dim across 8 cores (2 batches =
8192 tokens per core); W1h / W2 / b2 replicated. The vocab-row gather
(a pure indexed copy) and the hs0 transpose are folded into host-side
shard prep: each core receives
  - hsx [896, 8192] f32r : rows 0..767 = the hs0 shard transposed (the
        contraction dim must land on SBUF partitions; fp32 DMA-transpose
        doesn't exist on TRN2), rows 768..895 = (W1[:VOCAB]+b1)[tk].T
        (the vocab gather is a pure indexed copy -> shard prep),
  - w1x [128, 896] f32r : W1[VOCAB:] pre-chunked for lhsT loads, plus
        identity as chunk 6 (adds tok_part into the accumulation),
  - w2 [128, 1] f32r, b2 [1, 1] f32.

Per-core device kernel ("h.T layout", hs1 on partitions):
  - PSUM bank [128 hs1, 512 tok] accumulates 7 uniform f32r matmuls
    (full PE rate at moving free dim 512) contracting 896 rows.
  - relu on ACT -> SBUF (f32r), 128->1 contraction with W2 on PE,
    +b2 on DVE, small DMA out.
"""

import os

import numpy as np

HIDDEN = 768
VOCAB = 32000
HS1 = 128
B, S = 16, 4096
N_CORES = 8
T = (B // N_CORES) * S  # 8192 tokens per core
TB = 512  # tokens per DMA block
SUB = 512  # tokens per PE sub-block (PSUM bank width in f32)
N_HC = HIDDEN // 128  # 6 hidden chunks
N_C = N_HC + 1  # + tok chunk

_CACHE = {}


def _build_nc():
    import concourse.bacc as bacc
    import concourse.mybir as mybir
    import concourse.tile as tile

    f32 = mybir.dt.float32
    f32r = mybir.dt.float32r
    RELU = mybir.ActivationFunctionType.Relu

    nc = bacc.Bacc("TRN2", debug=False, target_bir_lowering=False)

    # blocked layout: block b's [896, TB] slab is contiguous in DRAM so
    # each block DMA is one sequential 1.75MB HBM read.
    hsx = nc.dram_tensor(
        "hsx", [(T // TB) * N_C * 128, TB], f32r, kind="ExternalInput"
    ).ap()
    w1x = nc.dram_tensor("w1x", [128, N_C * 128], f32r, kind="ExternalInput").ap()
    w2 = nc.dram_tensor("w2", [HS1, 1], f32r, kind="ExternalInput").ap()
    b2 = nc.dram_tensor("b2", [1, 1], f32, kind="ExternalInput").ap()
    out = nc.dram_tensor("out", [1, T], f32, kind="ExternalOutput").ap()

    with tile.TileContext(nc) as tc:
        with (
            tc.tile_pool(name="consts", bufs=1) as consts,
            tc.tile_pool(name="hs", bufs=6) as hs_pool,
            tc.tile_pool(name="hrelu", bufs=3) as h_pool,
            tc.tile_pool(name="osb", bufs=4) as o_pool,
            tc.tile_pool(name="ps", bufs=2, space="PSUM") as psum_pool,
            tc.tile_pool(name="ps2", bufs=2, space="PSUM") as ps2_pool,
        ):
            NB = T // TB
            hsx_r = hsx.rearrange("(b c p) t -> b p c t", c=N_C, p=128)

            # issue the first input blocks before the tiny const loads so
            # the big DMA stream starts as early as possible
            hxts = []
            def load_block(b):
                hxt = hs_pool.tile([128, N_C, TB], f32r, tag="hx", name=f"hx_{b}")
                nc.sync.dma_start(hxt[:], hsx_r[b, :, :, :])
                hxts.append(hxt)
            load_block(0)
            load_block(1)

            w1x_sb = consts.tile([128, N_C * 128], f32r)
            nc.sync.dma_start(w1x_sb[:], w1x[:])
            w2_sb = consts.tile([HS1, 1], f32r)
            nc.sync.dma_start(w2_sb[:], w2[:])
            b2_sb = consts.tile([1, 1], f32)
            nc.sync.dma_start(b2_sb[:], b2[:])

            deferred = []  # one-deep pipeline for the W2 dot + epilogue

            def epilogue(P, i):
                h = h_pool.tile([128, SUB], f32r, tag="h", name=f"h_{i}")
                nc.scalar.activation(h[:], P[:], RELU)
                P2 = ps2_pool.tile([1, SUB], f32, tag="P2", name=f"P2_{i}")
                nc.tensor.matmul(P2[:], w2_sb[:], h[:], start=True, stop=True)
                ot = o_pool.tile([1, SUB], f32, tag="ot", name=f"ot_{i}")
                nc.vector.tensor_scalar_add(ot[:], P2[:], b2_sb[:, :1])
                nc.sync.dma_start(out[:, i * SUB : (i + 1) * SUB], ot[:])

            for b in range(NB):
                if b + 2 < NB:
                    load_block(b + 2)
                hxt = hxts[b]
                for j in range(TB // SUB):
                    i = b * (TB // SUB) + j
                    ts = slice(j * SUB, (j + 1) * SUB)
                    P = psum_pool.tile([128, SUB], f32, tag="P", name=f"P_{i}")
                    for c in range(N_C):
                        nc.tensor.matmul(
                            P[:],
                            w1x_sb[:, c * 128 : (c + 1) * 128],
                            hxt[:, c, ts],
                            start=(c == 0),
                            stop=(c == N_C - 1),
                        )
                    if deferred:
                        epilogue(*deferred.pop())
                    deferred.append((P, i))
            epilogue(*deferred.pop())

    nc.compile()
    return nc


def _prep_shared(W1, b1, W2, b2):
    W1 = np.asarray(W1, dtype=np.float32)
    b1 = np.asarray(b1, dtype=np.float32)
    w1tok = W1[:VOCAB] + b1[None, :]
    w1h = W1[VOCAB:].reshape(N_HC, 128, HS1).transpose(1, 0, 2).reshape(128, N_HC * HS1)
    w1x = np.ascontiguousarray(
        np.concatenate([w1h, np.eye(128, dtype=np.float32)], axis=1)
    )
    w2 = np.ascontiguousarray(np.asarray(W2, dtype=np.float32).reshape(HS1, 1))
    b2 = np.asarray(b2, dtype=np.float32).reshape(1, 1)
    return w1tok, w1x, w2, b2


def _prep_core(tk, hs0, w1tok, c):
    nb = B // N_CORES
    tkc = np.asarray(tk[c * nb : (c + 1) * nb]).reshape(-1)
    hs = np.asarray(hs0[c * nb : (c + 1) * nb], dtype=np.float32).reshape(T, HIDDEN)
    hsx = np.empty((N_C * 128, T), dtype=np.float32)
    hsx[:HIDDEN] = hs.T
    hsx[HIDDEN:] = w1tok[tkc].T
    # block the layout: [(b c p), TB] so each block is contiguous
    hsx = np.ascontiguousarray(
        hsx.reshape(N_C * 128, T // TB, TB).transpose(1, 0, 2)
    ).reshape((T // TB) * N_C * 128, TB)
    return hsx


def kernel(tk, hs0, W1, b1, W2, b2):
    from concourse.bass_utils import run_bass_kernel_spmd

    if "nc" not in _CACHE:
        _CACHE["nc"] = _build_nc()
    nc = _CACHE["nc"]

    w1tok, w1x, w2, b2a = _prep_shared(W1, b1, W2, b2)
    in_maps = []
    for c in range(N_CORES):
        hsx = _prep_core(tk, hs0, w1tok, c)
        in_maps.append({"hsx": hsx, "w1x": w1x, "w2": w2, "b2": b2a})

    trace = bool(int(os.environ.get("KERNEL_TRACE", "0")))
    res = run_bass_kernel_spmd(
        nc, in_maps, core_ids=list(range(N_CORES)), trace=trace
    )
    _CACHE["last_results"] = res
    outs = [res.results[c]["out"].reshape(-1) for c in range(N_CORES)]
    return np.concatenate(outs).reshape(B, S).astype(np.float32)


# revision 17
# speedup vs baseline: 1.1057x; 1.1057x over previous
"""Trainium2 Bass kernel for the embedding_lookup Classifier problem.

Computation (per token t):
    out[t] = relu(W1[:VOCAB][tk[t]] + hs0[t] @ W1[VOCAB:] + b1) @ W2 + b2

Sharding: data-parallel over the batch dim across 8 cores (2 batches =
8192 tokens per core); W1h / W2 / b2 replicated. The vocab-row gather
(a pure indexed copy) and the hs0 transpose are folded into host-side
shard prep: each core receives
  - hsx [896, 8192] f32r : rows 0..767 = the hs0 shard transposed (the
        contraction dim must land on SBUF partitions; fp32 DMA-transpose
        doesn't exist on TRN2), rows 768..895 = (W1[:VOCAB]+b1)[tk].T
        (the vocab gather is a pure indexed copy -> shard prep),
  - w1x [128, 896] f32r : W1[VOCAB:] pre-chunked for lhsT loads, plus
        identity as chunk 6 (adds tok_part into the accumulation),
  - w2 [128, 1] f32r, b2 [1, 1] f32.

Per-core device kernel ("h.T layout", hs1 on partitions):
  - PSUM bank [128 hs1, 512 tok] accumulates 7 uniform f32r matmuls
    (full PE rate at moving free dim 512) contracting 896 rows.
  - relu on ACT -> SBUF (f32r), 128->1 contraction with W2 on PE,
    +b2 on DVE, small DMA out.
"""

import os

import numpy as np

HIDDEN = 768
VOCAB = 32000
HS1 = 128
B, S = 16, 4096
N_CORES = 8
T = (B // N_CORES) * S  # 8192 tokens per core
TB = 512  # tokens per DMA block
SUB = 512  # tokens per PE sub-block (PSUM bank width in f32)
N_HC = HIDDEN // 128  # 6 hidden chunks
N_C = N_HC + 1  # + tok chunk

_CACHE = {}


def _build_nc():
    import concourse.bacc as bacc
    import concourse.mybir as mybir
    import concourse.tile as tile

    f32 = mybir.dt.float32
    f32r = mybir.dt.float32r
    RELU = mybir.ActivationFunctionType.Relu

    nc = bacc.Bacc("TRN2", debug=False, target_bir_lowering=False)

    # blocked layout: block b's [896, TB] slab is contiguous in DRAM so
    # each block DMA is one sequential 1.75MB HBM read.
    hsx = nc.dram_tensor(
        "hsx", [(T // TB) * N_C * 128, TB], f32r, kind="ExternalInput"
    ).ap()
    w1x = nc.dram_tensor("w1x", [128, N_C * 128], f32r, kind="ExternalInput").ap()
    w2 = nc.dram_tensor("w2", [HS1, 1], f32r, kind="ExternalInput").ap()
    b2 = nc.dram_tensor("b2", [1, 1], f32, kind="ExternalInput").ap()
    out = nc.dram_tensor("out", [1, T], f32, kind="ExternalOutput").ap()

    with tile.TileContext(nc) as tc:
        with (
            tc.tile_pool(name="consts", bufs=1) as consts,
            tc.tile_pool(name="hs", bufs=8) as hs_pool,
            tc.tile_pool(name="hrelu", bufs=3) as h_pool,
            tc.tile_pool(name="osb", bufs=1) as o_pool,
            tc.tile_pool(name="ps", bufs=2, space="PSUM") as psum_pool,
            tc.tile_pool(name="ps2", bufs=2, space="PSUM") as ps2_pool,
        ):
            NB = T // TB
            hsx_r = hsx.rearrange("(b c p) t -> b p c t", c=N_C, p=128)

            # issue the first input blocks before the tiny const loads so
            # the big DMA stream starts as early as possible
            hxts = []
            def load_block(b):
                hxt = hs_pool.tile([128, N_C, TB], f32r, tag="hx", name=f"hx_{b}")
                nc.sync.dma_start(hxt[:], hsx_r[b, :, :, :])
                hxts.append(hxt)
            for _pb in range(4):
                load_block(_pb)

            w1x_sb = consts.tile([128, N_C * 128], f32r)
            nc.scalar.dma_start(w1x_sb[:], w1x[:])
            w2_sb = consts.tile([HS1, 1], f32r)
            nc.scalar.dma_start(w2_sb[:], w2[:])
            b2_sb = consts.tile([1, 1], f32)
            nc.scalar.dma_start(b2_sb[:], b2[:])

            out_sb = o_pool.tile([1, T], f32)

            deferred = []  # one-deep pipeline for the W2 dot + epilogue

            def epilogue(P, i):
                h = h_pool.tile([128, SUB], f32r, tag="h", name=f"h_{i}")
                nc.scalar.activation(h[:], P[:], RELU)
                P2 = ps2_pool.tile([1, SUB], f32, tag="P2", name=f"P2_{i}")
                nc.tensor.matmul(P2[:], w2_sb[:], h[:], start=True, stop=True)
                nc.vector.tensor_scalar_add(
                    out_sb[:, i * SUB : (i + 1) * SUB], P2[:], b2_sb[:, :1]
                )

            for b in range(NB):
                if b + 4 < NB:
                    load_block(b + 4)
                hxt = hxts[b]
                for j in range(TB // SUB):
                    i = b * (TB // SUB) + j
                    ts = slice(j * SUB, (j + 1) * SUB)
                    P = psum_pool.tile([128, SUB], f32, tag="P", name=f"P_{i}")
                    for c in range(N_C):
                        nc.tensor.matmul(
                            P[:],
                            w1x_sb[:, c * 128 : (c + 1) * 128],
                            hxt[:, c, ts],
                            start=(c == 0),
                            stop=(c == N_C - 1),
                        )
                    if deferred:
                        epilogue(*deferred.pop())
                    deferred.append((P, i))
            epilogue(*deferred.pop())
            nc.sync.dma_start(out[:], out_sb[:])

    nc.compile()
    return nc


def _prep_shared(W1, b1, W2, b2):
    W1 = np.asarray(W1, dtype=np.float32)
    b1 = np.asarray(b1, dtype=np.float32)
    w1tok = W1[:VOCAB] + b1[None, :]
    w1h = W1[VOCAB:].reshape(N_HC, 128, HS1).transpose(1, 0, 2).reshape(128, N_HC * HS1)
    w1x = np.ascontiguousarray(
        np.concatenate([w1h, np.eye(128, dtype=np.float32)], axis=1)
    )
    w2 = np.ascontiguousarray(np.asarray(W2, dtype=np.float32).reshape(HS1, 1))
    b2 = np.asarray(b2, dtype=np.float32).reshape(1, 1)
    return w1tok, w1x, w2, b2


def _prep_core(tk, hs0, w1tok, c):
    nb = B // N_CORES
    tkc = np.asarray(tk[c * nb : (c + 1) * nb]).reshape(-1)
    hs = np.asarray(hs0[c * nb : (c + 1) * nb], dtype=np.float32).reshape(T, HIDDEN)
    hsx = np.empty((N_C * 128, T), dtype=np.float32)
    hsx[:HIDDEN] = hs.T
    hsx[HIDDEN:] = w1tok[tkc].T
    # block the layout: [(b c p), TB] so each block is contiguous
    hsx = np.ascontiguousarray(
        hsx.reshape(N_C * 128, T // TB, TB).transpose(1, 0, 2)
    ).reshape((T // TB) * N_C * 128, TB)
    return hsx


def kernel(tk, hs0, W1, b1, W2, b2):
    from concourse.bass_utils import run_bass_kernel_spmd

    if "nc" not in _CACHE:
        _CACHE["nc"] = _build_nc()
    nc = _CACHE["nc"]

    w1tok, w1x, w2, b2a = _prep_shared(W1, b1, W2, b2)
    in_maps = []
    for c in range(N_CORES):
        hsx = _prep_core(tk, hs0, w1tok, c)
        in_maps.append({"hsx": hsx, "w1x": w1x, "w2": w2, "b2": b2a})

    trace = bool(int(os.environ.get("KERNEL_TRACE", "0")))
    res = run_bass_kernel_spmd(
        nc, in_maps, core_ids=list(range(N_CORES)), trace=trace
    )
    _CACHE["last_results"] = res
    outs = [res.results[c]["out"].reshape(-1) for c in range(N_CORES)]
    return np.concatenate(outs).reshape(B, S).astype(np.float32)
